# revision 5
# baseline (speedup 1.0000x reference)
"""Trainium2 8-core GQA causal attention kernel (Bass/Tile).

Problem: B=2, S=2048, D=4096, 32 Q heads / 8 KV heads, HD=128, RoPE
(interleaved pairs), causal mask, output projection.

Sharding: 8-way tensor parallel over KV-head groups. Core i owns query
heads 4i..4i+3 (wq cols i*512..), kv head i (wk/wv cols i*128..), and
OUTPUT columns i*512.. of wo.  Per core:
  qT = wq_i.T @ x.T        (512, T)   [all matmuls in transposed layout:
  kT = wk_i.T @ x.T        (128, T)    host passes xT so every operand is
  vT = wv_i.T @ x.T        (128, T)    native partition-major]
  RoPE on qT,kT: z*cosF + (Pswap z)*sinF2 (pair swap via PE permutation mm)
  v  = token-major via PE transpose of vT tiles
  S^T[tk,tq] = kT_tile.T @ qT_chunk; +mask on diagonal blocks;
  es = exp(S^T * scale) fused on ScalarE (PSUM->SBUF)
  out^T[c,tq] += v_tile @ es ; Z[tq] += ones @ es (replicated col-sums)
  out^T /= Z  -> outT (512, T) bf16
AllGather outT over 8 cores -> attnT (4096, T);
outP = wo_i.T @ attnT  (512 out cols, T); host concatenates + transposes.

Compute dtype bf16 (f32 accumulation in PSUM), storage f32 in/out.
"""
import sys
import numpy as np

sys.path.insert(0, "/opt/trn_rl_repo")

import ml_dtypes  # noqa: E402

BF16 = ml_dtypes.bfloat16

NCORES = 8
B, S, D = 2, 2048, 4096
H, KV, HD = 32, 8, 128
T = B * S
HPC = H // NCORES          # 4 query heads per core
CQ = HPC * HD              # 512
CKV = HD                   # 128
SC = 512                   # token chunk (free dim of moving operands)
ND = D // 128              # 32 contraction chunks
NT = T // SC               # 8 token chunks
SCALE = float(HD) ** -0.5


def host_prepare(x, cos, sin, mask, wq, wk, wv, wo):
    xM = np.ascontiguousarray(np.asarray(x, dtype=np.float32).reshape(T, D))
    xT = np.ascontiguousarray(xM.T).astype(BF16)                 # (D, T)
    cosF = np.repeat(np.asarray(cos, dtype=np.float32).T, 2, axis=0).astype(BF16)
    sinF2 = np.repeat(np.asarray(sin, dtype=np.float32).T, 2, axis=0)
    sinF2[0::2] *= -1.0
    sinF2 = sinF2.astype(BF16)                                   # (128, S)
    pswap = np.zeros((128, 128), dtype=np.float32)
    idx = np.arange(0, 128, 2)
    pswap[idx, idx + 1] = 1.0
    pswap[idx + 1, idx] = 1.0
    pswapT = pswap.astype(BF16)
    ident = np.eye(128, dtype=np.float32).astype(BF16)
    ones = np.ones((128, 128), dtype=np.float32).astype(BF16)
    maskT4 = np.concatenate(
        [np.asarray(mask, dtype=np.float32)[0:SC, r * 128:(r + 1) * 128].T
         for r in range(4)], axis=1
    ).astype(np.float32)                                         # (128, 4*SC)
    shared = dict(xT=xT, cosF=cosF, sinF2=sinF2, pswapT=pswapT, ident=ident,
                  ones=ones, maskT4=maskT4)
    cores = []
    for i in range(NCORES):
        cores.append(dict(
            wq=np.ascontiguousarray(wq[:, i * CQ:(i + 1) * CQ]).astype(BF16),
            wk=np.ascontiguousarray(wk[:, i * CKV:(i + 1) * CKV]).astype(BF16),
            wv=np.ascontiguousarray(wv[:, i * CKV:(i + 1) * CKV]).astype(BF16),
            wo=np.ascontiguousarray(wo[:, i * CQ:(i + 1) * CQ]).astype(BF16),
        ))
    return shared, cores


_CACHE = {}


def build_nc():
    from concourse import bass, bacc, mybir, tile

    F32 = mybir.dt.float32
    CDT = mybir.dt.bfloat16
    ADD = mybir.AluOpType.add
    MULT = mybir.AluOpType.mult
    EXP = mybir.ActivationFunctionType.Exp
    COPY = mybir.ActivationFunctionType.Copy

    nc = bacc.Bacc("TRN2", target_bir_lowering=False, debug=False,
                   num_devices=NCORES)

    def par(name, shape, dt, out=False):
        return nc.dram_tensor(name, shape, dt,
                              kind="ExternalOutput" if out else "ExternalInput").ap()

    xT_p = par("xT", [D, T], CDT)
    wq_p = par("wq", [D, CQ], CDT)
    wk_p = par("wk", [D, CKV], CDT)
    wv_p = par("wv", [D, CKV], CDT)
    wo_p = par("wo", [D, CQ], CDT)
    cos_p = par("cosF", [HD, S], CDT)
    sin_p = par("sinF2", [HD, S], CDT)
    psw_p = par("pswapT", [128, 128], CDT)
    idn_p = par("ident", [128, 128], CDT)
    one_p = par("ones", [128, 128], CDT)
    msk_p = par("maskT4", [128, 4 * SC], F32)
    out_p = par("out", [CQ, T], F32, out=True)

    def wload(dst, src, nblk, width):
        # dst sbuf [128, nblk*width] <- src dram [(nblk*128), width]
        for n in range(nblk):
            nc.sync.dma_start(
                out=dst[:, n * width:(n + 1) * width],
                in_=src[n * 128:(n + 1) * 128, :])

    with tile.TileContext(nc) as tc:
        with tc.tile_pool(name="res", bufs=1) as res, \
             tc.tile_pool(name="dram", bufs=1, space="DRAM") as dram:
            kT = res.tile([128, T], CDT, tag="kT")
            vsb = res.tile([128, T], CDT, tag="vsb")
            qT = res.tile([128, HPC * T], CDT, tag="qT")
            mk = res.tile([128, 4 * SC], F32, tag="mk")
            ones = res.tile([128, 128], CDT, tag="ones")
            nc.sync.dma_start(out=mk[:], in_=msk_p[:])
            nc.sync.dma_start(out=ones[:], in_=one_p[:])
            outT_d = dram.tile([CQ, T], CDT, tag="outT")
            agT_d = dram.tile([NCORES * CQ, T], CDT, tag="agT",
                              addr_space="Shared")

            # ---------------- phase 1: projections + rope + v transpose
            with tc.tile_pool(name="p1c", bufs=1) as p1c, \
                 tc.tile_pool(name="p1x", bufs=3) as p1x, \
                 tc.tile_pool(name="p1s", bufs=2) as p1s, \
                 tc.tile_pool(name="ps1", bufs=1, space="PSUM") as ps1, \
                 tc.tile_pool(name="ps1b", bufs=2, space="PSUM") as ps1b:
                wqs = p1c.tile([128, ND * CQ], CDT, tag="wqs")
                wks = p1c.tile([128, ND * CKV], CDT, tag="wks")
                wvs = p1c.tile([128, ND * CKV], CDT, tag="wvs")
                cosF = p1c.tile([128, S], CDT, tag="cosF")
                sinF2 = p1c.tile([128, S], CDT, tag="sinF2")
                psw = p1c.tile([128, 128], CDT, tag="psw")
                idn = p1c.tile([128, 128], CDT, tag="idn")
                wload(wqs, wq_p, ND, CQ)
                wload(wks, wk_p, ND, CKV)
                wload(wvs, wv_p, ND, CKV)
                nc.sync.dma_start(out=cosF[:], in_=cos_p[:])
                nc.sync.dma_start(out=sinF2[:], in_=sin_p[:])
                nc.sync.dma_start(out=psw[:], in_=psw_p[:])
                nc.sync.dma_start(out=idn[:], in_=idn_p[:])

                for tcn in range(NT):
                    t0 = tcn * SC
                    s0 = (tcn % (S // SC)) * SC
                    pq = [ps1.tile([128, SC], F32, tag=f"pq{h}", name=f"pq{h}")
                          for h in range(HPC)]
                    pk = ps1.tile([128, SC], F32, tag="pk")
                    pv = ps1.tile([128, SC], F32, tag="pv")
                    for d in range(ND):
                        xt = p1x.tile([128, SC], CDT, tag="xt")
                        nc.sync.dma_start(out=xt[:],
                                          in_=xT_p[d * 128:(d + 1) * 128,
                                                   t0:t0 + SC])
                        st = (d == 0)
                        sp = (d == ND - 1)
                        for h in range(HPC):
                            nc.tensor.matmul(
                                pq[h][:],
                                wqs[:, d * CQ + h * 128:d * CQ + (h + 1) * 128],
                                xt[:], start=st, stop=sp)
                        nc.tensor.matmul(pk[:],
                                         wks[:, d * CKV:(d + 1) * CKV],
                                         xt[:], start=st, stop=sp)
                        nc.tensor.matmul(pv[:],
                                         wvs[:, d * CKV:(d + 1) * CKV],
                                         xt[:], start=st, stop=sp)

                    # rope for q heads and k
                    for z in range(HPC + 1):
                        src = pq[z] if z < HPC else pk
                        raw = p1s.tile([128, SC], CDT, tag="raw")
                        nc.scalar.activation(raw[:], src[:], COPY)
                        psh = ps1b.tile([128, SC], F32, tag="psx", name="psh")
                        nc.tensor.matmul(psh[:], psw[:], raw[:],
                                         start=True, stop=True)
                        t1 = p1s.tile([128, SC], CDT, tag="t1")
                        nc.vector.tensor_tensor(t1[:], raw[:],
                                                cosF[:, s0:s0 + SC], MULT)
                        t2 = p1s.tile([128, SC], CDT, tag="t2")
                        nc.vector.tensor_tensor(t2[:], psh[:],
                                                sinF2[:, s0:s0 + SC], MULT)
                        if z < HPC:
                            dst = qT[:, z * T + t0:z * T + t0 + SC]
                        else:
                            dst = kT[:, t0:t0 + SC]
                        nc.vector.tensor_tensor(dst, t1[:], t2[:], ADD)

                    # v -> token-major via PE transpose
                    vts = p1s.tile([128, SC], CDT, tag="raw")
                    nc.scalar.activation(vts[:], pv[:], COPY)
                    for j in range(SC // 128):
                        ptr = ps1b.tile([128, 128], CDT, tag="psx", name="ptr")
                        nc.tensor.transpose(ptr[:],
                                            vts[:, j * 128:(j + 1) * 128],
                                            idn[:])
                        nc.scalar.activation(
                            vsb[:, t0 + j * 128:t0 + (j + 1) * 128],
                            ptr[:], COPY)

            # ---------------- phase 2: attention
            with tc.tile_pool(name="p2s", bufs=4) as p2s, \
                 tc.tile_pool(name="p2z", bufs=2) as p2z, \
                 tc.tile_pool(name="ps2", bufs=2, space="PSUM") as ps2, \
                 tc.tile_pool(name="ps2a", bufs=1, space="PSUM") as ps2a:
                for g in range(NT):
                    b, qj = divmod(g, S // SC)
                    nkb = (SC // 128) * (qj + 1)
                    for h in range(HPC):
                        po = ps2a.tile([128, SC], F32, tag="po", bufs=2)
                        pz = ps2a.tile([128, SC], F32, tag="pz", bufs=2)
                        rq = qT[:, h * T + g * SC:h * T + (g + 1) * SC]
                        for kb in range(nkb):
                            tkb = b * S + kb * 128
                            ps = ps2.tile([128, SC], F32, tag="ps")
                            nc.tensor.matmul(ps[:], kT[:, tkb:tkb + 128], rq,
                                             start=True, stop=True)
                            r = kb - qj * (SC // 128)
                            if r >= 0:
                                nc.vector.tensor_tensor(
                                    ps[:], ps[:], mk[:, r * SC:(r + 1) * SC],
                                    ADD)
                            es = p2s.tile([128, SC], CDT, tag="es")
                            nc.scalar.activation(es[:], ps[:], EXP,
                                                 scale=SCALE)
                            st = (kb == 0)
                            sp = (kb == nkb - 1)
                            nc.tensor.matmul(po[:], vsb[:, tkb:tkb + 128],
                                             es[:], start=st, stop=sp)
                            nc.tensor.matmul(pz[:], ones[:], es[:],
                                             start=st, stop=sp)
                        zr = p2z.tile([128, SC], F32, tag="zr")
                        nc.vector.reciprocal(zr[:], pz[:])
                        ot = p2z.tile([128, SC], CDT, tag="ot")
                        nc.vector.tensor_tensor(ot[:], po[:], zr[:], MULT)
                        nc.sync.dma_start(
                            out=outT_d[h * 128:(h + 1) * 128,
                                       g * SC:(g + 1) * SC],
                            in_=ot[:])

            # ---------------- all-gather
            nc.gpsimd.collective_compute(
                "AllGather", mybir.AluOpType.bypass,
                replica_groups=[list(range(NCORES))],
                ins=[outT_d.opt()], outs=[agT_d.opt()])

            # ---------------- phase 3: output projection (column shard)
            with tc.tile_pool(name="p3c", bufs=1) as p3c, \
                 tc.tile_pool(name="p3a", bufs=4) as p3a, \
                 tc.tile_pool(name="p3o", bufs=2) as p3o, \
                 tc.tile_pool(name="ps3", bufs=1, space="PSUM") as ps3:
                wos = p3c.tile([128, ND * CQ], CDT, tag="wos")
                wload(wos, wo_p, ND, CQ)
                NA = NCORES * CQ // 128  # 32 a-chunks
                for tt in range(NT):
                    po3 = [ps3.tile([128, SC], F32, tag=f"po3_{c}", name=f"po3_{c}")
                           for c in range(CQ // 128)]
                    for a in range(NA):
                        at = p3a.tile([128, SC], CDT, tag="at")
                        nc.sync.dma_start(
                            out=at[:],
                            in_=agT_d[a * 128:(a + 1) * 128,
                                      tt * SC:(tt + 1) * SC])
                        st = (a == 0)
                        sp = (a == NA - 1)
                        for c in range(CQ // 128):
                            nc.tensor.matmul(
                                po3[c][:],
                                wos[:, a * CQ + c * 128:a * CQ + (c + 1) * 128],
                                at[:], start=st, stop=sp)
                    for c in range(CQ // 128):
                        ob = p3o.tile([128, SC], F32, tag="ob")
                        nc.scalar.activation(ob[:], po3[c][:], COPY)
                        nc.sync.dma_start(
                            out=out_p[c * 128:(c + 1) * 128,
                                      tt * SC:(tt + 1) * SC],
                            in_=ob[:])

    nc.compile()
    return nc


def _run(inputs, trace=False, tmpdir=None):
    from concourse.bass_utils import run_bass_kernel_spmd

    if "nc" not in _CACHE:
        _CACHE["nc"] = build_nc()
    nc = _CACHE["nc"]
    shared, cores = host_prepare(
        inputs["x"], inputs["cos"], inputs["sin"], inputs["mask"],
        inputs["wq"], inputs["wk"], inputs["wv"], inputs["wo"])
    in_maps = []
    for i in range(NCORES):
        m = dict(shared)
        m.update(cores[i])
        in_maps.append(m)
    res = run_bass_kernel_spmd(nc, in_maps, list(range(NCORES)), trace=trace,
                               tmpdir=tmpdir)
    outs = [np.asarray(res.results[i]["out"], dtype=np.float32).T
            for i in range(NCORES)]                      # each (T, 512)
    full = np.concatenate(outs, axis=1).reshape(B, S, D)
    return full, res


def kernel(**inputs):
    out, _ = _run(inputs, trace=False)
    return out.astype(np.float32)


# revision 9
# speedup vs baseline: 1.3327x; 1.3327x over previous
"""Trainium2 8-core GQA causal attention kernel (Bass/Tile).

Problem: B=2, S=2048, D=4096, 32 Q heads / 8 KV heads, HD=128, RoPE
(interleaved pairs), causal mask, output projection.

Sharding: 8-way tensor parallel over KV-head groups. Core i owns query
heads 4i..4i+3 (wq cols i*512..), kv head i (wk/wv cols i*128..), and
OUTPUT columns i*512.. of wo.  Per core (all in transposed layout; the
host passes xT and tile-packed weights so every DMA is contiguous):
  qT = wq_i.T @ x.T ; kT = wk_i.T @ x.T ; vT = wv_i.T @ x.T
  RoPE: z*cosF + (Pswap z)*sinF2  (pair swap via PE permutation matmul)
  v -> token-major via PE transpose
  S^T[tk,tq] = kT_tile.T @ qT_chunk (+mask on diagonal blocks)
  es = exp(S^T * scale) fused on ScalarE (PSUM->SBUF)
  out^T[c,tq] += v_tile @ es ; Z[tq] += ones @ es (replicated col-sums)
  out^T = out^T / Z  -> outT chunk (bf16)
AllGather (4 token-range chunks, overlapped with attention) -> attnT;
outP = wo_i.T @ attnT  (512 out cols, T); host concatenates + transposes.
"""
import sys
import numpy as np

sys.path.insert(0, "/opt/trn_rl_repo")

import ml_dtypes  # noqa: E402

BF16 = ml_dtypes.bfloat16

NCORES = 8
B, S, D = 2, 2048, 4096
H, KV, HD = 32, 8, 128
T = B * S
HPC = H // NCORES          # 4 query heads per core
CQ = HPC * HD              # 512
CKV = HD                   # 128
SC = 512                   # token chunk (free dim of moving operands)
ND = D // 128              # 32 contraction chunks
NT = T // SC               # 8 token chunks
NA = NCORES * CQ // 128    # 32 attention-dim chunks in phase 3
NAG = 4                    # all-gather chunks (2 token chunks each)
SCALE = float(HD) ** -0.5


def _pack(a, width):
    """(n*128, width) -> (n, 128*width) tile-contiguous rows."""
    n = a.shape[0] // 128
    return np.ascontiguousarray(a.reshape(n, 128, width).reshape(n, 128 * width))


def host_prepare(x, cos, sin, mask, wq, wk, wv, wo):
    xM = np.ascontiguousarray(np.asarray(x, dtype=np.float32).reshape(T, D))
    xT = np.ascontiguousarray(xM.T).astype(BF16)                 # (D, T)
    # xTp[d*NT+t] = tile (d-chunk, t-chunk) flattened (128, SC)
    xTp = np.ascontiguousarray(
        xT.reshape(ND, 128, NT, SC).transpose(0, 2, 1, 3)
    ).reshape(ND * NT, 128 * SC)
    cosF = np.repeat(np.asarray(cos, dtype=np.float32).T, 2, axis=0).astype(BF16)
    sinF2 = np.repeat(np.asarray(sin, dtype=np.float32).T, 2, axis=0)
    sinF2[0::2] *= -1.0
    sinF2 = sinF2.astype(BF16)                                   # (128, S)
    pswap = np.zeros((128, 128), dtype=np.float32)
    idx = np.arange(0, 128, 2)
    pswap[idx, idx + 1] = 1.0
    pswap[idx + 1, idx] = 1.0
    pswapT = pswap.astype(BF16)
    ident = np.eye(128, dtype=np.float32).astype(BF16)
    ones = np.ones((128, 128), dtype=np.float32).astype(BF16)
    maskT4 = np.concatenate(
        [np.asarray(mask, dtype=np.float32)[0:SC, r * 128:(r + 1) * 128].T
         for r in range(4)], axis=1
    ).astype(np.float32)                                         # (128, 4*SC)
    shared = dict(xT=xTp, cosF=cosF, sinF2=sinF2, pswapT=pswapT, ident=ident,
                  ones=ones, maskT4=maskT4)
    cores = []
    for i in range(NCORES):
        cores.append(dict(
            wq=_pack(np.ascontiguousarray(wq[:, i * CQ:(i + 1) * CQ]).astype(BF16), CQ),
            wk=_pack(np.ascontiguousarray(wk[:, i * CKV:(i + 1) * CKV]).astype(BF16), CKV),
            wv=_pack(np.ascontiguousarray(wv[:, i * CKV:(i + 1) * CKV]).astype(BF16), CKV),
            wo=_pack(np.ascontiguousarray(wo[:, i * CQ:(i + 1) * CQ]).astype(BF16), CQ),
        ))
    return shared, cores


_CACHE = {}


def build_nc():
    from concourse import bacc, mybir, tile

    F32 = mybir.dt.float32
    CDT = mybir.dt.bfloat16
    ADD = mybir.AluOpType.add
    DIV = mybir.AluOpType.divide
    MULT = mybir.AluOpType.mult
    EXP = mybir.ActivationFunctionType.Exp
    COPY = mybir.ActivationFunctionType.Copy

    nc = bacc.Bacc("TRN2", target_bir_lowering=False, debug=False,
                   num_devices=NCORES)

    def par(name, shape, dt, out=False):
        return nc.dram_tensor(name, shape, dt,
                              kind="ExternalOutput" if out else "ExternalInput").ap()

    xT_p = par("xT", [ND * NT, 128 * SC], CDT)
    wq_p = par("wq", [ND, 128 * CQ], CDT)
    wk_p = par("wk", [ND, 128 * CKV], CDT)
    wv_p = par("wv", [ND, 128 * CKV], CDT)
    wo_p = par("wo", [ND, 128 * CQ], CDT)
    cos_p = par("cosF", [HD, S], CDT)
    sin_p = par("sinF2", [HD, S], CDT)
    psw_p = par("pswapT", [128, 128], CDT)
    idn_p = par("ident", [128, 128], CDT)
    one_p = par("ones", [128, 128], CDT)
    msk_p = par("maskT4", [128, 4 * SC], F32)
    # output: outP[c, t] packed as [(c/128)*NT + t-chunk, 128*SC]
    out_p = par("out", [(CQ // 128) * NT, 128 * SC], F32, out=True)

    xT_t = xT_p.rearrange("n (p c) -> n p c", p=128)
    out_t = out_p.rearrange("n (p c) -> n p c", p=128)

    with tile.TileContext(nc) as tc:
        with tc.tile_pool(name="res", bufs=1) as res, \
             tc.tile_pool(name="dram", bufs=1, space="DRAM") as dram:
            kT = res.tile([128, T], CDT, tag="kT")
            vsb = res.tile([128, T], CDT, tag="vsb")
            qT = res.tile([128, HPC * T], CDT, tag="qT")
            mk = res.tile([128, 4 * SC], F32, tag="mk")
            ones = res.tile([128, 128], CDT, tag="ones")
            nc.sync.dma_start(out=mk[:], in_=msk_p[:])
            nc.sync.dma_start(out=ones[:], in_=one_p[:])
            # packed outT rows: g-major (row = (g%2)*HPC + h within chunk)
            outT_d = [dram.tile([2 * HPC, 128 * SC], CDT, tag=f"outT{j}",
                                name=f"outT{j}") for j in range(NAG)]
            agT_d = [dram.tile([NCORES * 2 * HPC, 128 * SC], CDT,
                               tag=f"agT{j}", name=f"agT{j}",
                               addr_space="Shared") for j in range(NAG)]

            # ---------------- phase 1: projections + rope + v transpose
            with tc.tile_pool(name="p1c", bufs=1) as p1c, \
                 tc.tile_pool(name="p1x", bufs=4) as p1x, \
                 tc.tile_pool(name="p1s", bufs=3) as p1s, \
                 tc.tile_pool(name="p1r", bufs=7) as p1r, \
                 tc.tile_pool(name="ps1", bufs=1, space="PSUM") as ps1, \
                 tc.tile_pool(name="ps1b", bufs=2, space="PSUM") as ps1b:
                wqs = p1c.tile([128, ND * CQ], CDT, tag="wqs")
                wks = p1c.tile([128, ND * CKV], CDT, tag="wks")
                wvs = p1c.tile([128, ND * CKV], CDT, tag="wvs")
                cosF = p1c.tile([128, S], CDT, tag="cosF")
                sinF2 = p1c.tile([128, S], CDT, tag="sinF2")
                psw = p1c.tile([128, 128], CDT, tag="psw")
                idn = p1c.tile([128, 128], CDT, tag="idn")
                nc.sync.dma_start(out=cosF[:], in_=cos_p[:])
                nc.sync.dma_start(out=sinF2[:], in_=sin_p[:])
                nc.sync.dma_start(out=psw[:], in_=psw_p[:])
                nc.sync.dma_start(out=idn[:], in_=idn_p[:])
                # weights on the gpsimd queue so they don't block x tiles;
                # interleaved by d-chunk so d=0 of all three lands first.
                wq_t = wq_p.rearrange("n (p c) -> n p c", p=128)
                wk_t = wk_p.rearrange("n (p c) -> n p c", p=128)
                wv_t = wv_p.rearrange("n (p c) -> n p c", p=128)
                for d in range(ND):
                    nc.gpsimd.dma_start(out=wqs[:, d * CQ:(d + 1) * CQ],
                                        in_=wq_t[d])
                    nc.gpsimd.dma_start(out=wks[:, d * CKV:(d + 1) * CKV],
                                        in_=wk_t[d])
                    nc.gpsimd.dma_start(out=wvs[:, d * CKV:(d + 1) * CKV],
                                        in_=wv_t[d])

                for tcn in range(NT):
                    t0 = tcn * SC
                    s0 = (tcn % (S // SC)) * SC
                    pq = [ps1.tile([128, SC], F32, tag=f"pq{h}", name=f"pq{h}")
                          for h in range(HPC)]
                    pk = ps1.tile([128, SC], F32, tag="pk")
                    pv = ps1.tile([128, SC], F32, tag="pv")
                    for d in range(ND):
                        xt = p1x.tile([128, SC], CDT, tag="xt")
                        nc.sync.dma_start(out=xt[:], in_=xT_t[d * NT + tcn])
                        st = (d == 0)
                        sp = (d == ND - 1)
                        for h in range(HPC):
                            nc.tensor.matmul(
                                pq[h][:],
                                wqs[:, d * CQ + h * 128:d * CQ + (h + 1) * 128],
                                xt[:], start=st, stop=sp)
                        nc.tensor.matmul(pk[:],
                                         wks[:, d * CKV:(d + 1) * CKV],
                                         xt[:], start=st, stop=sp)
                        nc.tensor.matmul(pv[:],
                                         wvs[:, d * CKV:(d + 1) * CKV],
                                         xt[:], start=st, stop=sp)

                    # evict all psums first (frees banks for next chunk)
                    raws = []
                    for z in range(HPC + 2):
                        src = pq[z] if z < HPC else (pk if z == HPC else pv)
                        raw = p1r.tile([128, SC], CDT, tag="raw",
                                       name=f"raw{z}")
                        nc.scalar.activation(raw[:], src[:], COPY)
                        raws.append(raw)
                    # rope (q heads + k)
                    for z in range(HPC + 1):
                        raw = raws[z]
                        psh = ps1b.tile([128, SC], F32, tag="psx", name="psh")
                        nc.tensor.matmul(psh[:], psw[:], raw[:],
                                         start=True, stop=True)
                        t1 = p1s.tile([128, SC], CDT, tag="t1")
                        nc.vector.tensor_tensor(t1[:], raw[:],
                                                cosF[:, s0:s0 + SC], MULT)
                        t2 = p1s.tile([128, SC], CDT, tag="t2")
                        nc.vector.tensor_tensor(t2[:], psh[:],
                                                sinF2[:, s0:s0 + SC], MULT)
                        if z < HPC:
                            dst = qT[:, z * T + t0:z * T + t0 + SC]
                        else:
                            dst = kT[:, t0:t0 + SC]
                        nc.vector.tensor_tensor(dst, t1[:], t2[:], ADD)
                    # v -> token-major via PE transpose
                    vts = raws[HPC + 1]
                    for j in range(SC // 128):
                        ptr = ps1b.tile([128, 128], CDT, tag="psx", name="ptr")
                        nc.tensor.transpose(ptr[:],
                                            vts[:, j * 128:(j + 1) * 128],
                                            idn[:])
                        nc.scalar.activation(
                            vsb[:, t0 + j * 128:t0 + (j + 1) * 128],
                            ptr[:], COPY)

            # ---------------- phase 2: attention (+ chunked all-gather)
            with tc.tile_pool(name="p2s", bufs=18) as p2s, \
                 tc.tile_pool(name="p2z", bufs=2) as p2z, \
                 tc.tile_pool(name="ps2", bufs=4, space="PSUM") as ps2, \
                 tc.tile_pool(name="ps2a", bufs=2, space="PSUM") as ps2a:
                for g in range(NT):
                    b, qj = divmod(g, S // SC)
                    nkb = (SC // 128) * (qj + 1)
                    j, r = divmod(g, 2)
                    for h in range(HPC):
                        po = ps2a.tile([128, SC], F32, tag="po")
                        pz = ps2a.tile([128, SC], F32, tag="pz")
                        rq = qT[:, h * T + g * SC:h * T + (g + 1) * SC]
                        ess = []
                        for kb in range(nkb):
                            tkb = b * S + kb * 128
                            ps = ps2.tile([128, SC], F32, tag="ps",
                                          name=f"ps{kb}")
                            nc.tensor.matmul(ps[:], kT[:, tkb:tkb + 128], rq,
                                             start=True, stop=True)
                            rr = kb - qj * (SC // 128)
                            if rr >= 0:
                                nc.vector.tensor_tensor(
                                    ps[:], ps[:],
                                    mk[:, rr * SC:(rr + 1) * SC], ADD)
                            es = p2s.tile([128, SC], CDT, tag="es",
                                          name=f"es{kb}")
                            nc.scalar.activation(es[:], ps[:], EXP,
                                                 scale=SCALE)
                            ess.append((es, tkb))
                        for kb, (es, tkb) in enumerate(ess):
                            st = (kb == 0)
                            sp = (kb == nkb - 1)
                            nc.tensor.matmul(po[:], vsb[:, tkb:tkb + 128],
                                             es[:], start=st, stop=sp)
                            nc.tensor.matmul(pz[:], ones[:], es[:],
                                             start=st, stop=sp)
                        zr = p2z.tile([128, SC], F32, tag="zr")
                        nc.vector.reciprocal_approx_fast(zr[:], pz[:])
                        ot = p2z.tile([128, SC], CDT, tag="ot")
                        nc.vector.tensor_tensor(ot[:], po[:], zr[:], MULT)
                        nc.sync.dma_start(
                            out=outT_d[j].rearrange(
                                "n (p c) -> n p c", p=128)[r * HPC + h],
                            in_=ot[:])
                    if r == 1:
                        nc.gpsimd.collective_compute(
                            "AllGather", mybir.AluOpType.bypass,
                            replica_groups=[list(range(NCORES))],
                            ins=[outT_d[j].opt()], outs=[agT_d[j].opt()])

            # ---------------- phase 3: output projection (column shard)
            with tc.tile_pool(name="p3c", bufs=1) as p3c, \
                 tc.tile_pool(name="p3a", bufs=8) as p3a, \
                 tc.tile_pool(name="p3o", bufs=3) as p3o, \
                 tc.tile_pool(name="ps3", bufs=2, space="PSUM") as ps3:
                wos = p3c.tile([128, ND * CQ], CDT, tag="wos")
                wo_t = wo_p.rearrange("n (p c) -> n p c", p=128)
                for d in range(ND):
                    nc.gpsimd.dma_start(out=wos[:, d * CQ:(d + 1) * CQ],
                                        in_=wo_t[d])
                for g in range(NT):
                    j, r = divmod(g, 2)
                    agt = agT_d[j].rearrange("n (p c) -> n p c", p=128)
                    po3 = [ps3.tile([128, SC], F32, tag=f"po3_{c}",
                                    name=f"po3_{c}") for c in range(CQ // 128)]
                    for a in range(NA):
                        core, h = divmod(a, HPC)
                        at = p3a.tile([128, SC], CDT, tag="at")
                        nc.sync.dma_start(out=at[:],
                                          in_=agt[core * 2 * HPC + r * HPC + h])
                        st = (a == 0)
                        sp = (a == NA - 1)
                        for c in range(CQ // 128):
                            nc.tensor.matmul(
                                po3[c][:],
                                wos[:, a * CQ + c * 128:a * CQ + (c + 1) * 128],
                                at[:], start=st, stop=sp)
                    for c in range(CQ // 128):
                        ob = p3o.tile([128, SC], F32, tag="ob")
                        nc.scalar.activation(ob[:], po3[c][:], COPY)
                        nc.sync.dma_start(out=out_t[c * NT + g], in_=ob[:])

    nc.compile()
    return nc


def _run(inputs, trace=False, tmpdir=None):
    from concourse.bass_utils import run_bass_kernel_spmd

    if "nc" not in _CACHE:
        _CACHE["nc"] = build_nc()
    nc = _CACHE["nc"]
    shared, cores = host_prepare(
        inputs["x"], inputs["cos"], inputs["sin"], inputs["mask"],
        inputs["wq"], inputs["wk"], inputs["wv"], inputs["wo"])
    in_maps = []
    for i in range(NCORES):
        m = dict(shared)
        m.update(cores[i])
        in_maps.append(m)
    res = run_bass_kernel_spmd(nc, in_maps, list(range(NCORES)), trace=trace,
                               tmpdir=tmpdir)
    outs = []
    for i in range(NCORES):
        o = np.asarray(res.results[i]["out"], dtype=np.float32)
        # rows: (c/128)*NT + t-chunk, each 128*SC -> (CQ, T) -> (T, CQ)
        o = o.reshape(CQ // 128, NT, 128, SC).transpose(0, 2, 1, 3)
        outs.append(o.reshape(CQ, T).T)
    full = np.concatenate(outs, axis=1).reshape(B, S, D)
    return full, res


def kernel(**inputs):
    out, _ = _run(inputs, trace=False)
    return out.astype(np.float32)
